# revision 36
# baseline (speedup 1.0000x reference)
"""Trainium2 Bass kernel for nn_DMGCNLayer (GNN message passing layer).

Under this harness the end-to-end wall clock is dominated by the ~47 MB/s
axon host->device tunnel, so the design minimizes uploaded bytes first and
engine time second.

Strategy (graph/data parallel over 8 NeuronCores):
  - Edges are bucketed by dst node range (6250 nodes per core) so each core
    produces a disjoint slice of the output -> no cross-core reduction.
  - Within a core, edges are ordered by (128-node dst window, src%2) with
    uniform (max-over-cores) per-bucket budgets so that all 8 cores execute
    one identical SPMD program; shortfall is padded with self-neutralizing
    edges (their window-relative dst is 200, which matches no one-hot column).
  - eh is streamed per edge in fp8 e4m3 (validated ~8x inside the 2e-2
    rel-err budget).
  - h[src] is gathered ON DEVICE by the gpsimd ap_gather ucode from a
    bf16 [64, N/2, 2] node-pair table (only int16 indices are uploaded).
    Each (window, src%2) class run selects its pair slot with a stride-2
    copy, so no masks are needed. The table itself is uploaded as a 1/8
    fp8 shard per core and assembled with a device AllGather, then widened
    to bf16 on the scalar engine.
  - The per-edge MLP1 (m1 = relu(hs@Wn1+bn1)@Wn2) runs on the tensor engine.
  - h[dst] and the segment-sum are realized as one-hot matmuls on the tensor
    engine (edges are sorted by dst window), accumulating in fp32 PSUM. The
    [node, edge] one-hot for the hd gather is the PE-transpose (identity
    matmul) of the segment one-hot, so only the column layout of the
    window-relative dst is uploaded (bf16, widened to f32 on device).
  - The message MLPs run in transposed-activation form in bf16 with folded
    weights: m2 = relu(eh@(0.8 W_e1) + (hs*hd)@(0.2 W_ue@W_e1) + b_e1)@W_e2.
  - The aggregate leaves the device in fp8 (output absmax ~5.2, agg <= 0.9);
    the host adds the exact fp32 residual (+h) when assembling the output.
  - An on-device transpose-mode dma_gather path exists behind Cfg.use_gather
    but is disabled: that Q7 ucode faults under the axon/PJRT path
    (ap_gather does not).
"""

import math
from contextlib import ExitStack
from dataclasses import dataclass

import numpy as np
import ml_dtypes

import concourse.bass as bass
import concourse.bacc as bacc
import concourse.mybir as mybir
import concourse.tile as tile
from concourse import bass_utils

BF16 = ml_dtypes.bfloat16
FP8 = ml_dtypes.float8_e4m3
PADVAL = 200.0  # window-relative dst for pad edges; matches no iota column


@dataclass(frozen=True)
class Cfg:
    N: int = 50000
    E: int = 800000
    DN: int = 64
    H: int = 128
    NC: int = 8          # cores
    ST: int = 1024       # supertile (edges per pipeline step)
    CH: int = 8192       # edges per dma_gather call (multiple of ST)
    seg_oh_on_gpsimd: bool = False
    use_gather: bool = False
    use_apgather: bool = True   # gather h[src] on device via gpsimd ap_gather
    use_ag: bool = True         # shard node table, AllGather on device

    @property
    def NR(self):  # nodes per core
        return self.N // self.NC

    @property
    def NW(self):  # 128-node windows per core
        return -(-self.NR // 128)

    @property
    def SPLIT(self):  # src half split for int16 gather indices
        return self.N // 2


CFG_FULL = Cfg()


# --------------------------------------------------------------------------
# planning (uniform across cores)
# --------------------------------------------------------------------------

@dataclass
class Plan:
    budg: np.ndarray      # [2, NW] edge budget per (src-half, window), 128-mult
    pos0: np.ndarray      # [2, NW] start position of each bucket
    ET: int               # total positions per core (multiple of ST)
    calls: list           # [(pos0, n, half)]
    wchunk: np.ndarray    # [ET//128] window id of each 128-chunk
    first_chunk: np.ndarray  # [ET//128] bool: first chunk of its (half,win) block
    last_chunk: np.ndarray   # [ET//128] bool: last chunk of its (half,win) block
    budgc: np.ndarray | None = None    # [NW, 2] per-(window, src%2-class) budget
    runs: list | None = None           # [(gcol0, len, cls)] global class runs


def _make_plan(cfg: Cfg, src: np.ndarray, dst: np.ndarray) -> Plan:
    NR, NW = cfg.NR, cfg.NW
    core = dst // NR
    H2 = 2 if cfg.use_gather else 1
    half = (src >= cfg.SPLIT).astype(np.int64) if H2 == 2 else np.zeros(len(src), np.int64)
    win = (dst % NR) // 128

    budgc = None
    if cfg.use_apgather:
        assert H2 == 1
        cls = (src % 2).astype(np.int64)
        countsc = np.zeros((cfg.NC, NW, 2), np.int64)
        np.add.at(countsc, (core, win, cls), 1)
        budgc = countsc.max(axis=0)                  # [NW, 2]
        tot = budgc.sum(axis=1)
        pad = ((tot + 127) // 128) * 128 - tot
        budgc[:, 1] += pad                           # window pad -> class-1 tail
        budgc[NW - 1, 1] += (-budgc.sum()) % cfg.ST  # ST pad -> last window
        budg = budgc.sum(axis=1)[None, :]            # [1, NW]
    else:
        counts = np.zeros((cfg.NC, H2, NW), np.int64)
        np.add.at(counts, (core, half, win), 1)
        budg = counts.max(axis=0)
        budg = ((budg + 127) // 128) * 128
        # pad each half's total to a multiple of ST (grow the last window's
        # budget with pure-pad chunks; pads self-neutralize via one-hot miss)
        for hh in range(budg.shape[0]):
            budg[hh, NW - 1] += (-budg[hh].sum()) % cfg.ST

    H2 = budg.shape[0]
    pos0 = np.zeros((H2, NW), np.int64)
    off = 0
    for hh in range(H2):
        for w in range(NW):
            pos0[hh, w] = off
            off += budg[hh, w]
    ET = int(off)
    assert ET % cfg.ST == 0

    calls = []
    for hh in range(H2):
        h0 = int(pos0[hh, 0])
        hlen = int(budg[hh].sum())
        o = 0
        while o < hlen:
            n = min(cfg.CH, hlen - o)
            calls.append((h0 + o, n, hh))
            o += n

    runs = None
    if cfg.use_apgather:
        runs = []
        for w in range(NW):
            o = int(pos0[0, w])
            for c in range(2):
                ln = int(budgc[w, c])
                if ln:
                    runs.append((o, ln, c))
                    o += ln

    nch = ET // 128
    wchunk = np.zeros(nch, np.int64)
    first_chunk = np.zeros(nch, bool)
    last_chunk = np.zeros(nch, bool)
    for hh in range(H2):
        for w in range(NW):
            c0 = int(pos0[hh, w]) // 128
            c1 = c0 + int(budg[hh, w]) // 128
            wchunk[c0:c1] = w
            first_chunk[c0] = True
            last_chunk[c1 - 1] = True
    return Plan(budg, pos0, ET, calls, wchunk, first_chunk, last_chunk,
                budgc=budgc, runs=runs)


# --------------------------------------------------------------------------
# host-side input preparation
# --------------------------------------------------------------------------

def _prep(cfg: Cfg, inputs: dict, plan: Plan):
    h = np.asarray(inputs["h"], np.float32)
    eh = np.asarray(inputs["eh"], np.float32)
    src = np.asarray(inputs["src"]).astype(np.int64)
    dst = np.asarray(inputs["dst"]).astype(np.int64)
    W_node1 = np.asarray(inputs["W_node1"], np.float32)
    b_node1 = np.asarray(inputs["b_node1"], np.float32)
    W_node2 = np.asarray(inputs["W_node2"], np.float32)
    W_edge1 = np.asarray(inputs["W_edge1"], np.float32)
    b_edge1 = np.asarray(inputs["b_edge1"], np.float32)
    W_edge2 = np.asarray(inputs["W_edge2"], np.float32)
    W_comb = np.asarray(inputs["W_comb"], np.float32)
    W_ue = np.asarray(inputs["W_ue"], np.float32)

    NR, NW, ET = cfg.NR, cfg.NW, plan.ET

    hs_bf = h.astype(BF16)
    G1 = (np.maximum(h @ W_node1 + b_node1, 0.0) @ W_node2).astype(BF16)
    # record n = [hs(64) | G1_hi(64) | G1_lo(64) | 0(64)]; transpose-gather
    # puts hs on partitions 0:64 (slot 0), G1_hi on 64:128 (slot 0),
    # G1_lo on partitions 0:64 (slot 1) -- aligned with the m2 psum halves.
    recs = np.zeros((cfg.N, 256), BF16)
    recs[:, 0:64] = hs_bf
    recs[:, 64:128] = G1[:, 64:128]
    recs[:, 128:192] = G1[:, 0:64]

    # folded weights
    A = (0.8 * W_edge1).astype(BF16)                 # [64, H]
    W_ue1 = (0.2 * (W_ue @ W_edge1)).astype(BF16)    # [64, H]
    wzp = np.concatenate([W_ue1, A], axis=0)         # [128, H]; rows 0:64 act on p
    we2 = W_edge2.astype(BF16)
    wcomb = W_comb.astype(BF16)
    be1 = b_edge1.reshape(cfg.H, 1).astype(np.float32)
    iota_t = np.broadcast_to(np.arange(128, dtype=np.float32), (128, 128)).astype(BF16)
    iota_t = np.ascontiguousarray(iota_t)
    iota_c = np.arange(128, dtype=np.float32).reshape(128, 1)
    ones1 = np.ones((1, 128), BF16)

    core = dst // NR
    half = ((src >= cfg.SPLIT).astype(np.int64)
            if cfg.use_gather else np.zeros(len(src), np.int64))
    win = (dst % NR) // 128
    if cfg.use_apgather:
        assert cfg.N % 2 == 0 and cfg.N // 2 <= 32768
        cls = (src % 2).astype(np.int64)
        cls_off = np.zeros((NW, 2), np.int64)
        cls_off[:, 1] = plan.budgc[:, 0]
        htab = np.ascontiguousarray(h.T).astype(FP8)     # [64, N]

    in_maps = []
    for k in range(cfg.NC):
        # fill positions: bucket edges then pads
        perm = np.full(ET, -1, np.int64)
        mask_k = core == k
        ek = np.nonzero(mask_k)[0]
        # stable order by (half, win[, src%2 class])
        if cfg.use_apgather:
            key = win[ek] * 2 + cls[ek]
        else:
            key = half[ek] * NW + win[ek]
        order = np.argsort(key, kind="stable")
        ek = ek[order]
        key = key[order]
        # position of each edge: bucket start + rank within bucket
        if cfg.use_apgather:
            starts = plan.pos0[0, win[ek]] + cls_off[win[ek], cls[ek]]
        else:
            starts = plan.pos0[half[ek], win[ek]]
        # rank within bucket via cumcount on sorted keys
        changes = np.r_[True, key[1:] != key[:-1]]
        grp_start_idx = np.r_[0, np.nonzero(changes)[0][1:]]
        grp_id = np.cumsum(changes) - 1
        rank = np.arange(len(ek)) - grp_start_idx[grp_id]
        pos = starts + rank
        perm[pos] = ek

        valid = perm >= 0
        pe = perm[valid]

        eh_t = np.zeros((64, ET), FP8)
        eh_t[:, valid] = eh[pe].T.astype(FP8)

        if cfg.use_gather:
            # gather index values (relative to the half's base)
            idx_vals = np.zeros(ET, np.int16)
            sv = src[pe] - half[pe] * cfg.SPLIT
            assert sv.max(initial=0) < 32768
            idx_vals[valid] = sv.astype(np.int16)

        if cfg.use_gather:
            gidx16 = np.zeros((16, ET // 16), np.int16)
            for (p0, n, _hh) in plan.calls:
                blk = idx_vals[p0:p0 + n].reshape(n // 16, 16).T
                gidx16[:, p0 // 16:(p0 + n) // 16] = blk
            gidx = np.tile(gidx16, (8, 1))  # [128, ET//16]

        wrel = np.full(ET, PADVAL, np.float32)
        wrel[valid] = (dst[pe] - k * NR - win[pe] * 128).astype(np.float32)
        wrel_col = np.ascontiguousarray(
            wrel.reshape(ET // 128, 128).T).astype(BF16)  # [128, ET//128]
        wrel_row = wrel.reshape(1, ET).astype(BF16)

        hwin = np.zeros((128, NW * 64), FP8)
        hk = h[k * NR:(k + 1) * NR].astype(FP8)           # [NR, 64]
        for w in range(NW):
            rows = hk[w * 128:(w + 1) * 128]
            hwin[:rows.shape[0], w * 64:w * 64 + 64] = rows

        im = {
            "eh_t": eh_t,
            "wrel_col": wrel_col,
            "hwin": hwin,
            "wzp": wzp,
            "we2": we2,
            "wcomb": wcomb,
            "be1": be1,
            "iota_t": iota_t,
            "iota_c": iota_c,
            "wn1": W_node1.astype(BF16),
            "wn2": W_node2.astype(BF16),
            "bn1": b_node1.reshape(cfg.H, 1).astype(np.float32),
        }
        if not cfg.use_apgather:
            im["wrel_row"] = wrel_row
            im["ones1"] = ones1
        if cfg.use_gather:
            im["recs"] = recs
            im["gidx"] = gidx
        elif cfg.use_apgather:
            idx_vals = np.zeros(ET, np.int16)
            idx_vals[valid] = (src[pe] >> 1).astype(np.int16)
            im["gidx16"] = np.ascontiguousarray(
                idx_vals.reshape(ET // 16, 16).T)        # [16, ET//16]
            if cfg.use_ag:
                NSH = cfg.N // cfg.NC
                im["htab_sh"] = np.ascontiguousarray(
                    htab[:, k * NSH:(k + 1) * NSH])
            else:
                im["htab"] = htab
        else:
            sp = src[pe]
            hs_t = np.zeros((64, ET), FP8)
            hs_t[:, valid] = h[sp].T.astype(FP8)
            im["hs_t"] = hs_t
        in_maps.append(im)
    ctx = {"h": h}
    return in_maps, ctx


# --------------------------------------------------------------------------
# device program
# --------------------------------------------------------------------------

def _build(cfg: Cfg, plan: Plan) -> bacc.Bacc:
    ET, NW = plan.ET, cfg.NW
    f32 = mybir.dt.float32
    bf16 = mybir.dt.bfloat16
    fp8 = mybir.dt.float8e4
    i16 = mybir.dt.int16

    nc = bacc.Bacc("TRN2", target_bir_lowering=False, debug=False,
                   enable_asserts=False, num_devices=cfg.NC)

    d_eh = nc.dram_tensor("eh_t", [64, ET], fp8, kind="ExternalInput").ap()
    if cfg.use_gather:
        d_recs = nc.dram_tensor("recs", [cfg.N, 256], bf16, kind="ExternalInput").ap()
        d_gidx = nc.dram_tensor("gidx", [128, ET // 16], i16, kind="ExternalInput").ap()
    elif cfg.use_apgather:
        if cfg.use_ag:
            NSH = cfg.N // cfg.NC
            d_hsh = nc.dram_tensor("htab_sh", [64, NSH], fp8,
                                   kind="ExternalInput").ap()
            d_hshi = nc.dram_tensor("htab_shi", [64, NSH], fp8).ap()
            d_hgath = nc.dram_tensor("htab_gath", [cfg.NC * 64, NSH], fp8).ap()
        else:
            d_htab = nc.dram_tensor("htab", [64, cfg.N], fp8,
                                    kind="ExternalInput").ap()
        d_gidx16 = nc.dram_tensor("gidx16", [16, ET // 16], i16,
                                  kind="ExternalInput").ap()
    else:
        d_hst = nc.dram_tensor("hs_t", [64, ET], fp8, kind="ExternalInput").ap()
    d_wn1 = nc.dram_tensor("wn1", [64, cfg.H], bf16, kind="ExternalInput").ap()
    d_wn2 = nc.dram_tensor("wn2", [cfg.H, cfg.H], bf16, kind="ExternalInput").ap()
    d_bn1 = nc.dram_tensor("bn1", [cfg.H, 1], f32, kind="ExternalInput").ap()
    d_wrc = nc.dram_tensor("wrel_col", [128, ET // 128], bf16, kind="ExternalInput").ap()
    if not cfg.use_apgather:
        d_wrr = nc.dram_tensor("wrel_row", [1, ET], bf16, kind="ExternalInput").ap()
    d_hwin = nc.dram_tensor("hwin", [128, NW * 64], fp8, kind="ExternalInput").ap()
    d_wzp = nc.dram_tensor("wzp", [128, cfg.H], bf16, kind="ExternalInput").ap()
    d_we2 = nc.dram_tensor("we2", [cfg.H, cfg.H], bf16, kind="ExternalInput").ap()
    d_wcomb = nc.dram_tensor("wcomb", [cfg.H, 64], bf16, kind="ExternalInput").ap()
    d_be1 = nc.dram_tensor("be1", [cfg.H, 1], f32, kind="ExternalInput").ap()
    d_iota_t = nc.dram_tensor("iota_t", [128, 128], bf16, kind="ExternalInput").ap()
    d_iota_c = nc.dram_tensor("iota_c", [128, 1], f32, kind="ExternalInput").ap()
    if not cfg.use_apgather:
        d_ones1 = nc.dram_tensor("ones1", [1, 128], bf16, kind="ExternalInput").ap()
    d_agg = nc.dram_tensor("agg", [128, NW * 64], fp8, kind="ExternalOutput").ap()

    eq = mybir.AluOpType.is_equal
    mul = mybir.AluOpType.mult
    add = mybir.AluOpType.add
    Relu = mybir.ActivationFunctionType.Relu
    Tanh = mybir.ActivationFunctionType.Tanh

    NSTEP = ET // cfg.ST
    # map supertile -> (call index, local col offset)
    call_of_st = []
    for t in range(NSTEP):
        c0 = t * cfg.ST
        for ci, (p0, n, _hh) in enumerate(plan.calls):
            if p0 <= c0 < p0 + n:
                call_of_st.append((ci, c0 - p0))
                break
    assert len(call_of_st) == NSTEP

    with tile.TileContext(nc) as tc, ExitStack() as ctx:
        con = ctx.enter_context(tc.tile_pool(name="const", bufs=1))
        sb = ctx.enter_context(tc.tile_pool(name="sb", bufs=2))
        sohp = ctx.enter_context(tc.tile_pool(name="soh", bufs=12))
        gpool = ctx.enter_context(tc.tile_pool(name="gbuf", bufs=2))
        pers = ctx.enter_context(tc.tile_pool(name="pers", bufs=1))
        ps_a = ctx.enter_context(tc.tile_pool(name="ps_a", bufs=1, space="PSUM"))
        ps_b = ctx.enter_context(tc.tile_pool(name="ps_b", bufs=1, space="PSUM"))
        ps_hd = ctx.enter_context(tc.tile_pool(name="ps_hd", bufs=1, space="PSUM"))
        ps_bc = ctx.enter_context(tc.tile_pool(name="ps_bc", bufs=1, space="PSUM"))
        ps_mn = ctx.enter_context(tc.tile_pool(name="ps_mn", bufs=1, space="PSUM"))
        ps_ag = ctx.enter_context(tc.tile_pool(name="ps_ag", bufs=1, space="PSUM"))

        def load_const(tag, dram_ap, shape, dtype):
            t_ = con.tile(shape, dtype, tag=tag)
            nc.sync.dma_start(out=t_[:], in_=dram_ap)
            return t_

        c_wzp = load_const("wzp", d_wzp, [128, cfg.H], bf16)
        c_we2 = load_const("we2", d_we2, [cfg.H, cfg.H], bf16)
        c_wcomb = load_const("wcomb", d_wcomb, [cfg.H, 64], bf16)
        c_be1 = load_const("be1", d_be1, [cfg.H, 1], f32)
        c_iota_t = load_const("iota_t", d_iota_t, [128, 128], bf16)
        c_iota_c = load_const("iota_c", d_iota_c, [128, 1], f32)
        if cfg.use_apgather:
            # identity for PE transposes, built on device: (iota_t[p,c]==p)
            c_ident = con.tile([128, 128], bf16, tag="ident")
            nc.vector.tensor_scalar(c_ident[:], c_iota_t[:], c_iota_c[:],
                                    None, eq)
        else:
            c_ones1 = load_const("ones1", d_ones1, [1, 128], bf16)
        c_hwin8 = load_const("hwin", d_hwin, [128, NW * 64], fp8)
        c_hwin = con.tile([128, NW * 64], bf16, tag="hwin16")
        nc.vector.tensor_copy(out=c_hwin[:], in_=c_hwin8[:])
        if cfg.use_gather:
            c_gidx = load_const("gidx", d_gidx, [128, ET // 16], i16)
        c_wrcb = load_const("wrc", d_wrc, [128, ET // 128], bf16)
        c_wrc = con.tile([128, ET // 128], f32, tag="wrc32")
        nc.vector.tensor_copy(out=c_wrc[:], in_=c_wrcb[:])
        c_wn1 = load_const("wn1", d_wn1, [64, cfg.H], bf16)
        c_wn2 = load_const("wn2", d_wn2, [cfg.H, cfg.H], bf16)
        c_bn1 = load_const("bn1", d_bn1, [cfg.H, 1], f32)

        if cfg.use_apgather:
            # node table: DMA fp8 then widen to bf16 in chunks (staged)
            c_htab = pers.tile([64, cfg.N], bf16, tag="htab16")
            if cfg.use_ag:
                # each core uploads a 1/NC node-shard; AllGather assembles the
                # full table in DRAM, then stage+widen per shard block.
                # collectives may not touch IO tensors -> bounce via internal.
                # TileContext tracks the DRAM RAW deps across these.
                nc.sync.dma_start(out=d_hshi, in_=d_hsh)
                nc.gpsimd.collective_compute(
                    "AllGather", mybir.AluOpType.bypass,
                    replica_groups=[list(range(cfg.NC))],
                    ins=[d_hshi], outs=[d_hgath])
                NSH = cfg.N // cfg.NC
                for i in range(cfg.NC):
                    stg = gpool.tile([64, NSH], fp8, tag="htstg")
                    nc.sync.dma_start(out=stg[:],
                                      in_=d_hgath[64 * i:64 * (i + 1), :])
                    nc.scalar.activation(c_htab[:, i * NSH:(i + 1) * NSH],
                                         stg[:],
                                         mybir.ActivationFunctionType.Copy)
            else:
                TCH = min(cfg.N, 6250)
                assert cfg.N % TCH == 0
                for i in range(cfg.N // TCH):
                    stg = gpool.tile([64, TCH], fp8, tag="htstg")
                    nc.sync.dma_start(out=stg[:],
                                      in_=d_htab[:, i * TCH:(i + 1) * TCH])
                    nc.scalar.activation(c_htab[:, i * TCH:(i + 1) * TCH],
                                         stg[:],
                                         mybir.ActivationFunctionType.Copy)
            # gather indices, replicated into each 16-partition gpsimd group
            c_gidx = pers.tile([64, ET // 16], i16, tag="gidx")
            for g in range(4):
                nc.sync.dma_start(out=c_gidx[16 * g:16 * (g + 1), :],
                                  in_=d_gidx16)
            # class runs cut at supertile boundaries: per-t [(a, b, cls)]
            st_runs = [[] for _ in range(ET // cfg.ST)]
            for (o, ln, c) in plan.runs:
                a = o
                while a < o + ln:
                    t_ = a // cfg.ST
                    b = min((t_ + 1) * cfg.ST, o + ln)
                    st_runs[t_].append((a - t_ * cfg.ST, b - t_ * cfg.ST, c))
                    a = b

        agg_sb = pers.tile([128, NW * 64], fp8)
        aggp = ps_ag.tile([128, 8, 64], f32)  # rotating window accumulators

        gtiles = {}

        seg_eng = nc.gpsimd if cfg.seg_oh_on_gpsimd else nc.vector

        for t in range(NSTEP):
            if cfg.use_gather:
                ci, loc = call_of_st[t]
                if loc == 0:
                    p0, n, hh = plan.calls[ci]
                    gt = gpool.tile([128, 2, n], bf16, tag="gbuf")
                    src_ap = d_recs if hh == 0 else d_recs[cfg.SPLIT:, :]
                    nc.gpsimd.dma_gather(
                        out_ap=gt[:, :, :],
                        in_ap=src_ap,
                        idxs_ap=c_gidx[:, p0 // 16:(p0 + n) // 16],
                        num_idxs=n,
                        num_idxs_reg=n,
                        elem_size=256,
                        transpose=True,
                    )
                    gtiles[ci] = gt
                gt = gtiles[ci]
                hs_src = gt[0:64, 0, :]
                gofs = loc
            elif cfg.use_apgather:
                gath = gpool.tile([64, cfg.ST, 2], bf16, tag="gath")
                nc.gpsimd.ap_gather(
                    out_ap=gath[:, :, :],
                    in_ap=c_htab[:, :],
                    idxs_ap=c_gidx[:, t * (cfg.ST // 16):(t + 1) * (cfg.ST // 16)],
                    channels=64,
                    num_elems=cfg.N // 2,
                    d=2,
                    num_idxs=cfg.ST,
                )
                hs16 = sb.tile([64, cfg.ST], bf16, tag="hs16")
                for (a, b, c) in st_runs[t]:
                    nc.vector.tensor_copy(out=hs16[:, a:b],
                                          in_=gath[:, a:b, c])
                hs_src = hs16[:, :]
                gofs = 0
            else:
                hsb = gpool.tile([64, cfg.ST], fp8, tag="hst")
                nc.sync.dma_start(out=hsb[:],
                                  in_=d_hst[:, t * cfg.ST:(t + 1) * cfg.ST])
                hs16 = sb.tile([64, cfg.ST], bf16, tag="hs16")
                nc.scalar.activation(hs16[:], hsb[:],
                                     mybir.ActivationFunctionType.Copy)
                hs_src = hs16[:, :]
                gofs = 0

            # per-edge MLP1: m1 = relu(hs@Wn1 + bn1)@Wn2, in transposed form
            z1 = ps_a.tile([128, cfg.ST], f32, tag="za")
            for hhalf in range(cfg.ST // 512):
                cl0 = hhalf * 512
                nc.tensor.matmul(z1[:, cl0:cl0 + 512], c_wn1[:],
                                 hs_src[:, gofs + cl0:gofs + cl0 + 512],
                                 start=True, stop=True)
            r1 = sb.tile([128, cfg.ST], bf16, tag="r1")
            nc.vector.tensor_scalar(r1[:], z1[:], c_bn1[:, 0:1], 0.0,
                                    mybir.AluOpType.add, mybir.AluOpType.max)
            m1p = ps_b.tile([128, cfg.ST], f32, tag="zb")
            for hhalf in range(cfg.ST // 512):
                cl0 = hhalf * 512
                nc.tensor.matmul(m1p[:, cl0:cl0 + 512], c_wn2[:],
                                 r1[:, cl0:cl0 + 512], start=True, stop=True)
            m1sb = sb.tile([128, cfg.ST], bf16, tag="m1sb")
            nc.vector.tensor_copy(out=m1sb[:], in_=m1p[:])

            stack = sb.tile([128, cfg.ST], bf16, tag="stack")
            ehs = gpool.tile([64, cfg.ST], fp8, tag="ehs")
            nc.sync.dma_start(out=ehs[:],
                              in_=d_eh[:, t * cfg.ST:(t + 1) * cfg.ST])
            nc.scalar.activation(stack[64:128, :], ehs[:],
                                 mybir.ActivationFunctionType.Copy)
            if not cfg.use_apgather:
                wrr = sb.tile([1, cfg.ST], bf16, tag="wrr")
                nc.sync.dma_start(out=wrr[:],
                                  in_=d_wrr[:, t * cfg.ST:(t + 1) * cfg.ST])

            # per-128-chunk segment one-hot [edge, node-in-window]
            seg_ohs = []
            for j in range(cfg.ST // 128):
                c = t * (cfg.ST // 128) + j
                so = sohp.tile([128, 128], bf16, tag="soh")
                seg_eng.tensor_scalar(so[:], c_iota_t[:], c_wrc[:, c:c + 1],
                                      None, eq)
                seg_ohs.append(so)

            # hd via one-hot matmul, in 512-col halves
            for hhalf in range(cfg.ST // 512):
                cl0 = hhalf * 512
                j0 = cl0 // 128
                ohT = sb.tile([128, 512], bf16, tag="ohT")
                if cfg.use_apgather:
                    # [node, edge] one-hot = PE-transpose of the seg one-hots
                    ohTp = ps_bc.tile([128, 512], f32, tag="bc")
                    for j in range(j0, j0 + 4):
                        nc.tensor.matmul(ohTp[:, (j - j0) * 128:(j - j0 + 1) * 128],
                                         seg_ohs[j][:], c_ident[:],
                                         start=True, stop=True)
                    nc.vector.tensor_copy(out=ohT[:], in_=ohTp[:])
                else:
                    bc = ps_bc.tile([128, 512], f32, tag="bc")
                    nc.tensor.matmul(bc[:], c_ones1[:],
                                     wrr[:, cl0:cl0 + 512], start=True, stop=True)
                    nc.vector.tensor_scalar(ohT[:], bc[:], c_iota_c[:], None, eq)
                hd = ps_hd.tile([64, 512], f32, tag="hd")
                # window-parts inside this half (chunks are window-pure)
                j0 = cl0 // 128
                parts = []
                for j in range(j0, j0 + 4):
                    c = t * (cfg.ST // 128) + j
                    w = int(plan.wchunk[c])
                    if parts and parts[-1][2] == w:
                        parts[-1][1] += 128
                    else:
                        parts.append([j * 128 - cl0, 128, w])
                for (o, wd, w) in parts:
                    nc.tensor.matmul(hd[:, o:o + wd],
                                     c_hwin[:, w * 64:(w + 1) * 64],
                                     ohT[:, o:o + wd], start=True, stop=True)
                # p = hs * hd  -> stack partitions 0:64
                nc.vector.tensor_tensor(
                    out=stack[0:64, cl0:cl0 + 512],
                    in0=hs_src[:, gofs + cl0:gofs + cl0 + 512],
                    in1=hd[:, :], op=mul)

            z = ps_a.tile([128, cfg.ST], f32, tag="za")
            for hhalf in range(cfg.ST // 512):
                cl0 = hhalf * 512
                nc.tensor.matmul(z[:, cl0:cl0 + 512], c_wzp[:],
                                 stack[:, cl0:cl0 + 512], start=True, stop=True)
            rz = sb.tile([128, cfg.ST], bf16, tag="rz")
            nc.scalar.activation(rz[:], z[:], Relu, bias=c_be1[:, 0:1])

            m2 = ps_b.tile([128, cfg.ST], f32, tag="zb")
            for hhalf in range(cfg.ST // 512):
                cl0 = hhalf * 512
                nc.tensor.matmul(m2[:, cl0:cl0 + 512], c_we2[:],
                                 rz[:, cl0:cl0 + 512], start=True, stop=True)

            m2c = sb.tile([128, cfg.ST], bf16, tag="m2c")
            nc.scalar.activation(m2c[:], m2[:],
                                 mybir.ActivationFunctionType.Copy)
            q = sb.tile([128, cfg.ST], bf16, tag="q")
            q_eng = nc.vector if cfg.use_apgather else nc.gpsimd
            q_eng.tensor_tensor(out=q[:, :], in0=m1sb[:, :],
                                in1=m2c[:, :], op=mul)

            mnt = ps_mn.tile([128, cfg.ST // 128, 64], f32, tag="mnt")
            for j in range(cfg.ST // 128):
                nc.tensor.matmul(mnt[:, j, :], q[:, j * 128:(j + 1) * 128],
                                 c_wcomb[:], start=True, stop=True)
            msb = sb.tile([128, cfg.ST // 128, 64], bf16, tag="msb")
            nc.scalar.activation(msb[:], mnt[:], Tanh)

            for j in range(cfg.ST // 128):
                c = t * (cfg.ST // 128) + j
                w = int(plan.wchunk[c])
                first = bool(plan.first_chunk[c])
                last = bool(plan.last_chunk[c])
                slot = w % 8
                nc.tensor.matmul(aggp[:, slot, :], seg_ohs[j][:],
                                 msb[:, j, :], start=first, stop=last)
                if last:
                    # second pass over this window (src-half B) accumulates
                    c0 = int(plan.pos0[0, w]) // 128
                    is_first_pass = c == c0 + int(plan.budg[0, w]) // 128 - 1
                    if is_first_pass:
                        nc.vector.tensor_copy(out=agg_sb[:, w * 64:(w + 1) * 64],
                                              in_=aggp[:, slot, :])
                    else:
                        nc.vector.tensor_tensor(
                            out=agg_sb[:, w * 64:(w + 1) * 64],
                            in0=agg_sb[:, w * 64:(w + 1) * 64],
                            in1=aggp[:, slot, :], op=add)

        nc.sync.dma_start(out=d_agg, in_=agg_sb[:])

    nc.compile()
    return nc


# --------------------------------------------------------------------------
# entry points
# --------------------------------------------------------------------------

def _assemble(cfg: Cfg, results, ctx):
    h = ctx["h"]
    out = np.empty((cfg.N, cfg.DN), np.float32)
    for k in range(cfg.NC):
        agg = np.asarray(results[k]["agg"], np.float32)
        agg = agg.reshape(128, cfg.NW, 64).transpose(1, 0, 2).reshape(cfg.NW * 128, 64)
        out[k * cfg.NR:(k + 1) * cfg.NR] = agg[:cfg.NR] + h[k * cfg.NR:(k + 1) * cfg.NR]
    return out


def run_pipeline(cfg: Cfg, inputs: dict, backend: str = "hw", want_trace: bool = False):
    src = np.asarray(inputs["src"]).astype(np.int64)
    dst = np.asarray(inputs["dst"]).astype(np.int64)
    plan = _make_plan(cfg, src, dst)
    in_maps, ctx = _prep(cfg, inputs, plan)
    nc = _build(cfg, plan)
    if backend == "sim":
        from concourse.bass_interp import CoreSim
        results = []
        for k in range(cfg.NC):
            sim = CoreSim(nc, trace=False)
            for name, arr in in_maps[k].items():
                sim.tensor(name)[:] = arr
            sim.simulate()
            results.append({"agg": np.array(sim.tensor("agg"))})
        return _assemble(cfg, results, ctx), None
    res = bass_utils.run_bass_kernel_spmd(
        nc, in_maps, core_ids=list(range(cfg.NC)), trace=want_trace)
    return _assemble(cfg, res.results, ctx), res


def kernel(**inputs) -> np.ndarray:
    out, _ = run_pipeline(CFG_FULL, inputs, backend="hw")
    return out


if __name__ == "__main__":
    # smoke test at small scale on the simulator
    cfg = Cfg(N=2048, E=8192, NC=2, ST=1024, CH=2048, use_ag=False)
    rng = np.random.default_rng(0)
    inputs = {
        "h": rng.standard_normal((cfg.N, 64), np.float32),
        "eh": rng.standard_normal((cfg.E, 64), np.float32),
        "W_node1": rng.standard_normal((64, 128), np.float32) * 0.05,
        "b_node1": rng.standard_normal((128,), np.float32) * 0.05,
        "W_node2": rng.standard_normal((128, 128), np.float32) * 0.05,
        "W_edge1": rng.standard_normal((64, 128), np.float32) * 0.05,
        "b_edge1": rng.standard_normal((128,), np.float32) * 0.05,
        "W_edge2": rng.standard_normal((128, 128), np.float32) * 0.05,
        "W_comb": rng.standard_normal((128, 64), np.float32) * 0.05,
        "W_ue": rng.standard_normal((64, 64), np.float32) * 0.05,
        "src": rng.integers(0, cfg.N, cfg.E).astype(np.int32),
        "dst": rng.integers(0, cfg.N, cfg.E).astype(np.int32),
    }
    h, eh = inputs["h"], inputs["eh"]
    hs, hd = h[inputs["src"]], h[inputs["dst"]]
    eh_new = 0.8 * eh + 0.2 * ((hs * hd) @ inputs["W_ue"])
    m1 = np.maximum(hs @ inputs["W_node1"] + inputs["b_node1"], 0) @ inputs["W_node2"]
    m2 = np.maximum(eh_new @ inputs["W_edge1"] + inputs["b_edge1"], 0) @ inputs["W_edge2"]
    m = np.tanh((m1 * m2) @ inputs["W_comb"])
    agg = np.zeros((cfg.N, 64), np.float32)
    np.add.at(agg, inputs["dst"], m)
    expected = agg + h

    out, _ = run_pipeline(cfg, inputs, backend="sim")
    err = np.abs(out - expected)
    rel = np.abs(err).max() / np.abs(expected).max()
    print("max abs err:", err.max(), " rel(absmax):", rel)
    print("mean abs err:", err.mean())
    assert rel < 2e-2, "accuracy failure"
    print("SIM OK")

